# revision 1
# baseline (speedup 1.0000x reference)
"""Sliding-window attention Trainium2 Bass kernel.

Problem: B=4, H=32, L=4096, D=128, window=512.
reference: attends over the LAST w=512 key/value positions; query row i may
only see window slot j when j <= i (slots are key positions L-w+j).

Sharding: B*H = 128 (b,h) pairs split across 8 cores -> 16 heads/core.
Pure data parallelism, no collectives.

Per-head on-device algorithm (all matmuls f32r, 1 cycle/row):
  S^T chunks [wc=128, qg] = (K^T chunk)^T . dot(Q^T group)   (PE, PSUM)
  mask-add on the first 512 queries                          (DVE)
  P^T = exp(S^T * 1/sqrt(D))                                 (ACT, one pass/group)
  rowsum[1, qg] = ones^T @ P^T chunks (accumulated)          (PE)
  recip = 1/rowsum                                           (DVE)
  bcast recip to 128 partitions                              (GPSIMD)
  O^T [D, qg] += V_c^T @ P^T chunks                          (PE)
  out = O^T * recip_bcast                                    (DVE, PSUM->SBUF)
Host pre-transposes Q/K to [head, D, L] layout and post-transposes the
[head, D, L] output back to [B, H, L, D].
"""

import math
from contextlib import ExitStack

import numpy as np

N_CORES = 8
B, H, L, D = 4, 32, 4096, 128
W = 512            # window
HEADS_PER_CORE = (B * H) // N_CORES   # 16
QG = 256           # queries per group (group = 2 query tiles of 128)
NCHUNK = W // 128  # 4 window chunks
NEG = -1.0e9       # additive mask value (pre-scale)
SCALE = 1.0 / math.sqrt(D)

_COMPILED = None


def _build():
    import concourse.tile as tile
    from concourse import bacc, mybir

    nc = bacc.Bacc("TRN2", target_bir_lowering=False, debug=False,
                   num_devices=N_CORES)

    f32r = mybir.dt.float32r
    f32 = mybir.dt.float32

    qT = nc.dram_tensor("qT", [HEADS_PER_CORE, D, L], f32r, kind="ExternalInput").ap()
    kT = nc.dram_tensor("kT", [HEADS_PER_CORE, D, W], f32r, kind="ExternalInput").ap()
    v = nc.dram_tensor("v", [HEADS_PER_CORE, W, D], f32r, kind="ExternalInput").ap()
    maskT = nc.dram_tensor("maskT", [W, W], f32, kind="ExternalInput").ap()
    ones = nc.dram_tensor("ones", [128, 1], f32r, kind="ExternalInput").ap()
    outT = nc.dram_tensor("outT", [HEADS_PER_CORE, D, L], f32, kind="ExternalOutput").ap()

    n_groups = L // QG
    masked_groups = W // QG  # first groups of each head that need the causal mask

    with tile.TileContext(nc) as tc:
        with ExitStack() as ctx:
            const = ctx.enter_context(tc.tile_pool(name="const", bufs=1))
            kt_pool = ctx.enter_context(tc.tile_pool(name="kt", bufs=2))
            v_pool = ctx.enter_context(tc.tile_pool(name="v", bufs=2))
            q_pool = ctx.enter_context(tc.tile_pool(name="q", bufs=2))
            o_pool = ctx.enter_context(tc.tile_pool(name="o", bufs=2))
            p_pool = ctx.enter_context(tc.tile_pool(name="p", bufs=3))
            recip_pool = ctx.enter_context(tc.tile_pool(name="recip", bufs=3))
            rbc_pool = ctx.enter_context(tc.tile_pool(name="rbc", bufs=3))
            s_psum = ctx.enter_context(tc.tile_pool(name="s_ps", bufs=2, space="PSUM"))
            o_psum = ctx.enter_context(tc.tile_pool(name="o_ps", bufs=2, space="PSUM"))
            sum_psum = ctx.enter_context(tc.tile_pool(name="sum_ps", bufs=2, space="PSUM"))

            # core-resident constants
            mask_t = const.tile([128, NCHUNK * W], f32, tag="mask")
            for c in range(NCHUNK):
                nc.gpsimd.dma_start(mask_t[:, c * W:(c + 1) * W],
                                    maskT[c * 128:(c + 1) * 128, :])
            ones_t = const.tile([128, 1], f32r, tag="ones")
            nc.gpsimd.dma_start(ones_t[:], ones[:])

            for h in range(HEADS_PER_CORE):
                kt_t = kt_pool.tile([128, W], f32r, tag="kt")
                nc.gpsimd.dma_start(kt_t[:], kT[h])
                v_t = v_pool.tile([128, NCHUNK * D], f32r, tag="v")
                for c in range(NCHUNK):
                    nc.gpsimd.dma_start(v_t[:, c * D:(c + 1) * D],
                                        v[h, c * 128:(c + 1) * 128, :])
                qt_t = q_pool.tile([128, L], f32r, tag="q")
                for i in range(4):
                    nc.gpsimd.dma_start(qt_t[:, i * (L // 4):(i + 1) * (L // 4)],
                                        qT[h, :, i * (L // 4):(i + 1) * (L // 4)])
                o_t = o_pool.tile([128, L], f32, tag="o")

                for g in range(n_groups):
                    qs = slice(g * QG, (g + 1) * QG)
                    s_ps = s_psum.tile([128, NCHUNK * QG], f32, tag="s")
                    for c in range(NCHUNK):
                        nc.tensor.matmul(
                            s_ps[:, c * QG:(c + 1) * QG],
                            lhsT=kt_t[:, c * 128:(c + 1) * 128],
                            rhs=qt_t[:, qs],
                            start=True, stop=True,
                        )
                    if g < masked_groups:
                        for c in range(NCHUNK):
                            # skip chunks whose mask slice is identically zero
                            if c * 128 + 127 > g * QG:
                                nc.vector.tensor_add(
                                    s_ps[:, c * QG:(c + 1) * QG],
                                    s_ps[:, c * QG:(c + 1) * QG],
                                    mask_t[:, c * W + g * QG:c * W + (g + 1) * QG],
                                )
                    p_t = p_pool.tile([128, NCHUNK * QG], f32r, tag="p")
                    nc.scalar.activation(p_t[:], s_ps[:],
                                         mybir.ActivationFunctionType.Exp,
                                         scale=SCALE)
                    sums_ps = sum_psum.tile([1, QG], f32, tag="sums")
                    for c in range(NCHUNK):
                        nc.tensor.matmul(
                            sums_ps[:],
                            lhsT=ones_t[:],
                            rhs=p_t[:, c * QG:(c + 1) * QG],
                            start=(c == 0), stop=(c == NCHUNK - 1),
                        )
                    recip_t = recip_pool.tile([1, QG], f32, tag="recip")
                    nc.vector.reciprocal(recip_t[:], sums_ps[:])
                    rbc_t = rbc_pool.tile([128, QG], f32, tag="rbc")
                    nc.gpsimd.partition_broadcast(rbc_t[:], recip_t[:])
                    o_ps = o_psum.tile([128, QG], f32, tag="ops")
                    for c in range(NCHUNK):
                        nc.tensor.matmul(
                            o_ps[:],
                            lhsT=v_t[:, c * D:(c + 1) * D],
                            rhs=p_t[:, c * QG:(c + 1) * QG],
                            start=(c == 0), stop=(c == NCHUNK - 1),
                        )
                    nc.vector.tensor_mul(o_t[:, qs], o_ps[:], rbc_t[:])

                for i in range(4):
                    nc.gpsimd.dma_start(outT[h, :, i * (L // 4):(i + 1) * (L // 4)],
                                        o_t[:, i * (L // 4):(i + 1) * (L // 4)])

    nc.compile()
    return nc


def _get_compiled():
    global _COMPILED
    if _COMPILED is None:
        _COMPILED = _build()
    return _COMPILED


def kernel(query, keys, values, window_size):
    from concourse.bass_utils import run_bass_kernel_spmd

    q = np.asarray(query, dtype=np.float32)
    k = np.asarray(keys, dtype=np.float32)
    v = np.asarray(values, dtype=np.float32)
    w = int(window_size)
    assert q.shape == (B, H, L, D) and w == W, (q.shape, w)

    nc = _get_compiled()

    # host-side prep: flatten (b,h), pre-transpose to [head, D, L]
    qf = q.reshape(B * H, L, D)
    kf = k.reshape(B * H, L, D)[:, L - W:, :]
    vf = v.reshape(B * H, L, D)[:, L - W:, :]

    # additive mask in S^T layout: maskT[j, i] = NEG where query i < slot j
    mT = np.where(np.arange(W)[None, :] < np.arange(W)[:, None],
                  np.float32(NEG), np.float32(0.0))
    ones = np.ones((128, 1), dtype=np.float32)

    in_maps = []
    for core in range(N_CORES):
        s = slice(core * HEADS_PER_CORE, (core + 1) * HEADS_PER_CORE)
        in_maps.append({
            "qT": np.ascontiguousarray(qf[s].transpose(0, 2, 1)),
            "kT": np.ascontiguousarray(kf[s].transpose(0, 2, 1)),
            "v": np.ascontiguousarray(vf[s]),
            "maskT": mT,
            "ones": ones,
        })

    res = run_bass_kernel_spmd(nc, in_maps, core_ids=list(range(N_CORES)))

    outs = [res.results[c]["outT"].transpose(0, 2, 1) for c in range(N_CORES)]
    return np.concatenate(outs, axis=0).reshape(B, H, L, D)
